# revision 15
# baseline (speedup 1.0000x reference)
"""DeepseekV3-style MoE block on 8 Trainium2 NeuronCores.

Strategy (expert-parallel, host-side dispatch/combine):
  - Router (sigmoid + top-2 + normalize) computed on host in fp32. The
    routing decides the sharding itself: tokens are gathered per expert on
    the host (the "all-to-all dispatch") and each core e runs expert e's
    SwiGLU FFN over its gathered token batch (padded to a common capacity).
  - Shared expert: tensor-parallel 2-way over the hidden dim (HS=1024 ->
    two 512 halves) x data-parallel 4-way over tokens. Core e computes the
    ws-half (e // 4) over token slice (e % 4). The two halves of each token
    slice are summed on the host.
  - Combine: host scatter-adds routed outputs (scaled by routing weights)
    and adds shared partials.

Device kernel (identical program on all 8 cores): two SwiGLU FFN
evaluations [ntok,1024]x[1024,512] -> silu*mul -> [ntok,512]x[512,1024].
Everything is kept feature-major (features on SBUF partitions, tokens on
the free axis) so no transposes are needed:
    hT[h,t]  = sum_d W1[d,h] * xT[d,t]      (lhsT=W1 chunk, rhs=xT chunk)
    gT[h,t]  = silu(h1T) * h3T
    y[t,d]   = sum_h gT[h,t] * W2[h,d]      (lhsT=gT chunk,  rhs=W2 chunk)

PRECISION:
  - "f32r": fp32 data on the wire; matmuls in float32r (TF32-like PE
    fast path, 1 cyc/row at free-dim >=256). rel err ~2.6e-4.
  - "bf16": weights+activations cast to bf16 on the host; fp32 PSUM
    accumulate. Half the DMA bytes, full PE rate. rel err ~4e-3.
  - "f32": exact fp32 matmuls (4 cyc/row). rel err ~1e-6.
"""

import os
import sys
from contextlib import ExitStack

import numpy as np

if "/opt/trn_rl_repo" not in sys.path and not os.path.isdir(
    os.path.join(os.path.dirname(os.path.abspath(__file__)), "concourse")
):
    sys.path.append("/opt/trn_rl_repo")

D = 1024  # model dim
E = 8  # experts
K = 2  # top-k
H = 512  # expert hidden
HS = 1024  # shared hidden
N_CORES = 8
TP_SHARED = 2  # shared expert split over HS
DP_SHARED = N_CORES // TP_SHARED  # shared expert split over tokens

PRECISION = os.environ.get("MOE_PRECISION", "f32r")

_NC_CACHE = {}
LAST_RUN = None  # BassKernelResults of the most recent kernel() call


def _build_nc(cap, ts):
    """One-core Bass/Tile program: routed FFN over `cap` tokens + shared
    FFN half over `ts` tokens.

    DRAM inputs (host-prepared, feature-major):
      xT   [D, cap+ts]  routed-gathered tokens | shared token slice
      w13  [D, 2H]      expert w1 | w3 columns
      w2e  [H, D]
      vs13 [D, 2H]      shared-half ws1 | ws3 columns
      vs2  [H, D]
    Outputs: yr [cap, D] (unscaled routed), ys [ts, D] (shared partial).
    """
    import concourse.bacc as bacc
    import concourse.mybir as mybir
    import concourse.tile as tile

    f32 = mybir.dt.float32
    f32r = mybir.dt.float32r
    bf16 = mybir.dt.bfloat16
    AF = mybir.ActivationFunctionType

    wire = bf16 if PRECISION == "bf16" else f32

    def mm(ap):
        return ap.bitcast(f32r) if PRECISION == "f32r" else ap

    nc = bacc.Bacc("TRN2", target_bir_lowering=False)

    xT = nc.declare_dram_parameter("xT", [D, cap + ts], wire, isOutput=False)
    w13 = nc.declare_dram_parameter("w13", [D, 2 * H], wire, isOutput=False)
    w2e = nc.declare_dram_parameter("w2e", [H, D], wire, isOutput=False)
    vs13 = nc.declare_dram_parameter("vs13", [D, 2 * H], wire, isOutput=False)
    vs2 = nc.declare_dram_parameter("vs2", [H, D], wire, isOutput=False)
    yr = nc.declare_dram_parameter("yr", [cap, D], f32, isOutput=True)
    ys = nc.declare_dram_parameter("ys", [ts, D], f32, isOutput=True)

    KC = D // 128  # contraction chunks for the first matmul
    HC = H // 128  # hidden chunks

    with ExitStack() as ctx:
        tc = ctx.enter_context(tile.TileContext(nc))
        wpool = ctx.enter_context(tc.tile_pool(name="w", bufs=1))
        xpool = ctx.enter_context(tc.tile_pool(name="x", bufs=1))
        gpool = ctx.enter_context(tc.tile_pool(name="g", bufs=2))
        spool = ctx.enter_context(tc.tile_pool(name="s", bufs=4))
        ypool = ctx.enter_context(tc.tile_pool(name="y", bufs=3))
        hps = ctx.enter_context(tc.tile_pool(name="hps", bufs=3, space="PSUM"))
        yps = ctx.enter_context(tc.tile_pool(name="yps", bufs=2, space="PSUM"))

        def load_chunk(pool, dram, c, ncol, pfx):
            t = pool.tile([128, ncol], wire, tag=f"{pfx}{c}", name=f"{pfx}{c}")
            nc.sync.dma_start(mm(t[:]), mm(dram[c * 128 : (c + 1) * 128, :]))
            return t

        # Loads emitted in first-use order so the first matmuls start as
        # early as possible: (w13[dc], xT[dc]) pairs, then w2, then shared.
        w13_t = []
        xT_t = []
        for dc in range(KC):
            w13_t.append(load_chunk(wpool, w13, dc, 2 * H, "w13_"))
            xT_t.append(load_chunk(xpool, xT, dc, cap + ts, "xt"))
        w2_t = [load_chunk(wpool, w2e, hc, D, "w2_") for hc in range(HC)]
        vs13_t = [load_chunk(wpool, vs13, dc, 2 * H, "v13_") for dc in range(KC)]
        vs2_t = [load_chunk(wpool, vs2, hc, D, "v2_") for hc in range(HC)]

        def token_groups(ntok):
            """Split ntok (a multiple of 128) into free-dim groups <=512,
            each a multiple of 128 and (when possible) >=256 so float32r
            matmuls run at full rate."""
            n512, r = divmod(ntok, 512)
            gs = [512] * n512
            if r == 128 and n512 >= 1:
                gs[-1] = 384
                r = 256
            if r:
                gs.append(r)
            return gs

        def ffn(wa_t, w2a_t, out_dram, tok0, ntok, pfx):
            """SwiGLU FFN over xT[:, tok0:tok0+ntok] with w13-style packed
            first-layer weights `wa_t` and second-layer `w2a_t`."""
            g0 = 0
            for F in token_groups(ntok):
                a0 = tok0 + g0
                gs = []
                for hc in range(HC):
                    h1 = hps.tile([128, 512], f32, tag="h1", name="h1")
                    for dc in range(KC):
                        nc.tensor.matmul(
                            h1[:, :F],
                            mm(wa_t[dc][:, hc * 128 : (hc + 1) * 128]),
                            mm(xT_t[dc][:, a0 : a0 + F]),
                            start=(dc == 0),
                            stop=(dc == KC - 1),
                        )
                    h3 = hps.tile([128, 512], f32, tag="h3", name="h3")
                    for dc in range(KC):
                        nc.tensor.matmul(
                            h3[:, :F],
                            mm(wa_t[dc][:, H + hc * 128 : H + (hc + 1) * 128]),
                            mm(xT_t[dc][:, a0 : a0 + F]),
                            start=(dc == 0),
                            stop=(dc == KC - 1),
                        )
                    s1 = spool.tile([128, 512], f32, tag="s1", name="s1")
                    nc.scalar.activation(s1[:, :F], h1[:, :F], AF.Silu)
                    g = gpool.tile(
                        [128, 512], wire, tag=f"g{hc}", name=f"g{pfx}{hc}"
                    )
                    nc.vector.tensor_mul(mm(g[:, :F]), s1[:, :F], h3[:, :F])
                    gs.append(g)
                for mt in range(F // 128):
                    r0 = g0 + mt * 128
                    y_sb = ypool.tile([128, D], f32, tag="ysb", name="ysb")
                    for nh in range(2):
                        yp = yps.tile([128, 512], f32, tag="yp", name="yp")
                        for hc in range(HC):
                            nc.tensor.matmul(
                                yp[:],
                                mm(gs[hc][:, mt * 128 : (mt + 1) * 128]),
                                mm(w2a_t[hc][:, nh * 512 : (nh + 1) * 512]),
                                start=(hc == 0),
                                stop=(hc == HC - 1),
                            )
                        if nh == 0:
                            nc.scalar.activation(y_sb[:, 0:512], yp[:], AF.Copy)
                        else:
                            nc.vector.tensor_copy(y_sb[:, 512:1024], yp[:])
                    nc.sync.dma_start(out_dram[r0 : r0 + 128, :], y_sb[:])
                g0 += F

        ffn(w13_t, w2_t, yr, 0, cap, "r")
        ffn(vs13_t, vs2_t, ys, cap, ts, "s")

    nc.compile()
    return nc


def kernel(x, gate_w, w1, w3, w2, ws1, ws3, ws2):
    global LAST_RUN
    from concourse.bass_utils import run_bass_kernel_spmd

    x = np.asarray(x, dtype=np.float32)
    gate_w = np.asarray(gate_w, dtype=np.float32)
    w1 = np.asarray(w1, dtype=np.float32)
    w3 = np.asarray(w3, dtype=np.float32)
    w2 = np.asarray(w2, dtype=np.float32)
    ws1 = np.asarray(ws1, dtype=np.float32)
    ws3 = np.asarray(ws3, dtype=np.float32)
    ws2 = np.asarray(ws2, dtype=np.float32)

    if PRECISION == "bf16":
        import ml_dtypes

        wire_np = ml_dtypes.bfloat16
    else:
        wire_np = np.float32

    b, s, d = x.shape
    T = b * s
    xt = np.ascontiguousarray(x.reshape(T, d))
    ts = T // DP_SHARED  # shared-expert token slice per DP group

    # ---- Router on host (fp32, matches the jax reference's selection) ----
    logits = xt @ gate_w  # [T, E]
    with np.errstate(over="ignore"):
        scores = 1.0 / (1.0 + np.exp(-logits, dtype=np.float32))
    top2 = np.argpartition(-scores, 1, axis=1)[:, :2]  # top-2 set per token
    rows = np.arange(T)
    sel_scores = scores[rows[:, None], top2]  # [T, 2]
    norm_w = sel_scores / sel_scores.sum(axis=1, keepdims=True)

    tok_ids = []
    tok_w = []
    sel = np.zeros((T, E), dtype=bool)
    wmat = np.zeros((T, E), dtype=np.float32)
    sel[rows[:, None], top2] = True
    wmat[rows[:, None], top2] = norm_w
    for e in range(E):
        ids = np.nonzero(sel[:, e])[0]
        tok_ids.append(ids)
        tok_w.append(wmat[ids, e])

    max_ne = max(len(ids) for ids in tok_ids)
    cap = max(128, -(-max_ne // 128) * 128)

    # ---- Build per-core shards ----
    xtT = np.ascontiguousarray(xt.T).astype(wire_np)  # [D, T]
    in_maps = []
    for e in range(E):
        ids = tok_ids[e]
        sl = e % DP_SHARED
        hf = e // DP_SHARED
        xT = np.zeros((d, cap + ts), dtype=wire_np)
        xT[:, : len(ids)] = xtT[:, ids]
        xT[:, cap:] = xtT[:, sl * ts : (sl + 1) * ts]
        w13 = np.concatenate([w1[e], w3[e]], axis=1).astype(wire_np)
        vs13 = np.concatenate(
            [ws1[:, hf * H : (hf + 1) * H], ws3[:, hf * H : (hf + 1) * H]],
            axis=1,
        ).astype(wire_np)
        in_maps.append(
            {
                "xT": xT,
                "w13": np.ascontiguousarray(w13),
                "w2e": np.ascontiguousarray(w2[e]).astype(wire_np),
                "vs13": np.ascontiguousarray(vs13),
                "vs2": np.ascontiguousarray(
                    ws2[hf * H : (hf + 1) * H, :]
                ).astype(wire_np),
            }
        )

    key = (cap, ts, PRECISION)
    nc = _NC_CACHE.get(key)
    if nc is None:
        nc = _build_nc(cap, ts)
        _NC_CACHE[key] = nc

    LAST_RUN = run_bass_kernel_spmd(nc, in_maps, list(range(N_CORES)))
    results = LAST_RUN.results

    # ---- Combine on host ----
    out = np.zeros((T, d), dtype=np.float32)
    for e in range(E):
        ids = tok_ids[e]
        out[ids] += results[e]["yr"][: len(ids)] * tok_w[e][:, None]
        sl = e % DP_SHARED
        out[sl * ts : (sl + 1) * ts] += results[e]["ys"]
    return out.reshape(b, s, d)
